# revision 1
# baseline (speedup 1.0000x reference)
"""Chamfer loss (nn_ChamferLoss) Trainium2 Bass kernel.

Problem: x, y: [B=4, D=3, N=M=8192] fp32. Output: scalar
    dist = mean_b mean_n min_m d2[b,n,m] + mean_b mean_m min_n d2[b,n,m]
    d2 = |x_n|^2 + |y_m|^2 - 2 x_n.y_m

Strategy
--------
* Host: pre-round points to the PE's f32r format and augment to 7 dims so a
  single K=7 f32r matmul (1 cyc/row) emits exact squared distances between
  the rounded points:
    xa = [-2*xr, |xr|^2_hi, |xr|^2_lo, 1, 1]
    ya = [ yr,   1,         1,         |yr|^2_hi, |yr|^2_lo]
* Sharding: 8 cores = 4 batches x 2 halves of N. Each core owns a
  [4096, 8192] distance block.
* Per core, loop column groups (2048 wide) outer, row tiles (128) inner:
    PE    : 4 f32r matmuls -> PSUM [128,2048] per chunk
    ACT   : convert PSUM fp32 -> SBUF fp16 *negated* (scale=-1), so all
            mins become maxes (gpsimd partition_all_reduce has max, not min)
    DVE   : tensor_scalar(max) w/ accum_out = fused row-max per chunk (4x),
            plus two interleaved column-accum chains (2x tensor_tensor max)
    POOL  : group-end partition_all_reduce(max) over partitions
  Host: negate, combine core pairs, final means.
"""

import numpy as np
from contextlib import ExitStack

import concourse.bass_isa as bass_isa
import concourse.mybir as mybir
import concourse.tile as tile
from concourse import bacc
from concourse.bass_utils import run_bass_kernel_spmd

B, D, N, M = 4, 3, 8192, 8192
NCORES = 8
NHALF = N // 2            # rows per core
P = 128                   # partitions
NT = NHALF // P           # 32 row tiles per core
MT = 512                  # matmul moving free size (one PSUM bank fp32)
CHUNK = 2048              # per-chunk width (4 matmul tiles, 4 PSUM banks)
NG = M // CHUNK           # 4 column groups
KA = 7                    # augmented contraction dim (hi/lo norm splits)

F32 = mybir.dt.float32
F32R = mybir.dt.float32r
F16 = mybir.dt.float16

BIG = 3.0e38
# row tiles whose negate+convert+row-max runs as ONE fused DVE tensor_scalar
# (op0=mult(-1) from PSUM, op1=max accum) instead of ACT convert + DVE TSP.
# NOTE: plain TensorTensor is NOT legal on the Pool engine (walrus rejects
# it on TRN2), so both column-accum chains run on DVE; Pool only does the
# partition_all_reduce tails.
FUSED_CONV = frozenset({1, 5, 9, 13, 17, 21, 25})
# row tiles whose column-max is taken directly by a Pool partition_all_reduce
# on the conv tile (skipping the DVE chain); their [1,CHUNK] partials ship to
# the host, which max-combines all partial rows per group.
POOL_RED = frozenset({2, 3, 6, 7, 10, 11, 14, 15, 18, 19, 22, 23, 26, 30})
NPART = 2 + len(POOL_RED)   # partial col-max rows per group

_cached_nc = None
last_results = None


def _build():
    """Build and compile the per-core SPMD program (same on all 8 cores)."""
    global _cached_nc
    if _cached_nc is not None:
        return _cached_nc

    nc = bacc.Bacc("TRN2", target_bir_lowering=False, debug=False,
                   num_devices=NCORES)

    xt = nc.dram_tensor("xt", [KA, NHALF], F32R, kind="ExternalInput").ap()
    yt = nc.dram_tensor("yt", [KA, M], F32R, kind="ExternalInput").ap()
    # negated row maxes: [p, t] ; negated col maxes: [g, j]
    rowres_d = nc.dram_tensor("rowres", [P, NT], F32, kind="ExternalOutput").ap()
    # partial col-max rows per group (2 chains + Pool-reduced tiles);
    # host max-combines them
    colres_d = nc.dram_tensor("colres", [NG, NPART, CHUNK], F16,
                              kind="ExternalOutput").ap()

    mx = mybir.AluOpType.max

    with tile.TileContext(nc) as tc, ExitStack() as ctx:
        consts = ctx.enter_context(tc.tile_pool(name="consts", bufs=1))
        accs = ctx.enter_context(tc.tile_pool(name="accs", bufs=1))
        conv_pool = ctx.enter_context(tc.tile_pool(name="conv", bufs=8))
        cacc_pool = ctx.enter_context(tc.tile_pool(name="cacc", bufs=2))
        psum_pool = ctx.enter_context(
            tc.tile_pool(name="psum", bufs=2, space="PSUM"))

        xs = consts.tile([KA, NHALF], F32R)
        nc.sync.dma_start(out=xs[:], in_=xt)
        ys = consts.tile([KA, M], F32R)
        for gd in range(NG):   # split so the first matmul starts sooner
            sl = slice(gd * CHUNK, (gd + 1) * CHUNK)
            nc.sync.dma_start(out=ys[:, sl], in_=yt[:, sl])

        rmin_all = accs.tile([P, NT * NG], F32)   # accum slot per (t, g)
        rowres = accs.tile([P, NT], F32)
        # tiny dummy ACT op: pulls the Copy act-table load into the DMA wait
        nc.gpsimd.memset(rowres[:, 0:1], 0.0)
        nc.scalar.mul(rowres[:, 0:1], rowres[:, 0:1], 0.0)

        for g in range(NG):
            cacc_a = cacc_pool.tile([P, CHUNK], F16, tag="cacc_a")
            cacc_b = cacc_pool.tile([P, CHUNK], F16, tag="cacc_b")
            for t in range(NT):
                lhsT = xs[:, t * P:(t + 1) * P]          # [KA, 128] f32r
                ps = psum_pool.tile([P, CHUNK], F32, tag="ps")
                for j in range(CHUNK // MT):
                    m0 = g * CHUNK + j * MT
                    nc.tensor.matmul(
                        ps[:, j * MT:(j + 1) * MT], lhsT,
                        ys[:, m0:m0 + MT], start=True, stop=True)
                conv = conv_pool.tile([P, CHUNK], F16, tag="conv")
                fused = t in FUSED_CONV
                if fused:   # one DVE op: negate+convert+row-max accum
                    nc.vector.tensor_scalar(
                        conv[:], ps[:], -1.0, None,
                        op0=mybir.AluOpType.mult, op1=mx,
                        accum_out=rmin_all[:, t * NG + g:t * NG + g + 1])
                else:       # negate+convert on ACT
                    nc.scalar.mul(conv[:], ps[:], -1.0)
                # column-max: Pool-reduced tiles skip the DVE chains
                if t == 0:
                    nc.vector.tensor_copy(cacc_a[:], conv[:])
                elif t == 1:
                    nc.vector.tensor_copy(cacc_b[:], conv[:])
                elif t not in POOL_RED:
                    if t % 2 == 0:
                        nc.vector.tensor_tensor(cacc_a[:], cacc_a[:], conv[:],
                                                op=mx)
                    else:
                        nc.vector.tensor_tensor(cacc_b[:], cacc_b[:], conv[:],
                                                op=mx)
                # row-max of this chunk (DVE 4x mode), one slot per (t,g)
                if not fused:
                    nc.vector.tensor_scalar(
                        conv[:], conv[:], -BIG, None, op0=mx, op1=mx,
                        accum_out=rmin_all[:, t * NG + g:t * NG + g + 1])
                if t in POOL_RED:   # direct col-max of this tile on POOL
                    nc.gpsimd.partition_all_reduce(conv[:], conv[:], P,
                                                   bass_isa.ReduceOp.max)
                    slot = 2 + sorted(POOL_RED).index(t)
                    nc.sync.dma_start(out=colres_d[g, slot:slot + 1, :],
                                      in_=conv[0:1, :])
            # partition-reduce each chain on POOL; host max-combines them
            nc.gpsimd.partition_all_reduce(cacc_a[:], cacc_a[:], P,
                                           bass_isa.ReduceOp.max)
            nc.gpsimd.partition_all_reduce(cacc_b[:], cacc_b[:], P,
                                           bass_isa.ReduceOp.max)
            nc.sync.dma_start(out=colres_d[g, 0:1, :], in_=cacc_a[0:1, :])
            nc.sync.dma_start(out=colres_d[g, 1:2, :], in_=cacc_b[0:1, :])

        nc.vector.tensor_reduce(
            rowres[:], rmin_all[:].rearrange("p (t g) -> p t g", g=NG),
            axis=mybir.AxisListType.X, op=mx)
        nc.sync.dma_start(out=rowres_d, in_=rowres[:])

    nc.compile()
    _cached_nc = nc
    return nc


def _f32r_round(a):
    """Round fp32 to the PE's f32r format: 1s + 8e + 11m (top 20 bits), RNE."""
    u = np.ascontiguousarray(a, np.float32).view(np.uint32).astype(np.uint64)
    lsb = (u >> 12) & 1
    u = ((u + 0x7FF + lsb) >> 12) << 12
    return (u & 0xFFFFFFFF).astype(np.uint32).view(np.float32)


def _augment(x, y):
    """Host-side augmentation. x,y: [B, 3, N] fp32 -> xa,ya: [B, 7, *] f32r.

    Points are pre-rounded to f32r so the PE computes the exact squared
    distance between the *rounded* points: |xr|^2 is computed from xr and
    carried as f32r hi + residual lo rows (both exactly representable up
    to ~1e-7), preserving the |xr-yr|^2 cancellation structure.
    """
    xr = _f32r_round(x)
    yr = _f32r_round(y)
    ones = np.ones((x.shape[0], 1, x.shape[2]), np.float32)

    def hilo(sq):
        hi = _f32r_round(sq)
        lo = _f32r_round(sq - hi)
        return hi[:, None, :], lo[:, None, :]

    xsq_hi, xsq_lo = hilo(np.sum(xr * xr, axis=1, dtype=np.float32))
    ysq_hi, ysq_lo = hilo(np.sum(yr * yr, axis=1, dtype=np.float32))
    xa = np.concatenate([-2.0 * xr, xsq_hi, xsq_lo, ones, ones],
                        axis=1).astype(np.float32)
    ya = np.concatenate([yr, ones, ones, ysq_hi, ysq_lo],
                        axis=1).astype(np.float32)
    return xa, ya


def kernel(x, y):
    global last_results
    x = np.ascontiguousarray(np.asarray(x, dtype=np.float32))
    y = np.ascontiguousarray(np.asarray(y, dtype=np.float32))
    assert x.shape == (B, D, N) and y.shape == (B, D, M)

    xa, ya = _augment(x, y)

    in_maps = []
    for c in range(NCORES):
        b, h = divmod(c, 2)
        in_maps.append({
            "xt": np.ascontiguousarray(xa[b, :, h * NHALF:(h + 1) * NHALF]),
            "yt": np.ascontiguousarray(ya[b]),
        })

    nc = _build()
    res = run_bass_kernel_spmd(nc, in_maps, list(range(NCORES)))
    last_results = res

    cham_x = 0.0
    cham_y = 0.0
    for b in range(B):
        r0 = res.results[2 * b]
        r1 = res.results[2 * b + 1]
        # rowres holds max(-d2) = -min(d2) per row
        row_sum = -(r0["rowres"].astype(np.float64).sum()
                    + r1["rowres"].astype(np.float64).sum())
        # colres holds per-half, per-chain max(-d2) per column; combine all
        colmax = np.maximum(r0["colres"], r1["colres"]).max(axis=1)
        col_sum = -colmax.astype(np.float64).sum()
        cham_x += row_sum / N
        cham_y += col_sum / M
    dist = cham_x / B + cham_y / B
    return np.float32(dist)

